# revision 1
# baseline (speedup 1.0000x reference)
"""Trainium2 Bass kernel for an AttentionBlock (GroupNorm -> 1x1-conv QKV ->
softmax attention -> 1x1-conv projection -> residual).

Sharding: 8 cores = (batch b in 0..3) x (half of the h*w=4096 query positions).
Each core receives the full x[b] (needed for GroupNorm stats and for K/V over
all positions) plus its 2048-column query slice, and produces the [64, 2048]
output slice.  All cores run the identical SPMD program.

Key algorithmic choices (per core):
  - GroupNorm is folded into per-channel scale/shift A, B computed on-chip
    (free-dim reduce for per-channel sum/sumsq, tiny PE matmul against a group
    indicator matrix for the cross-partition group aggregation, rstd via
    exp(-0.5*ln(var+eps)) so only one ACT table set is ever needed; a dummy Ln
    at kernel start overlaps the table load with the input DMA).
  - Attention uses the S^T layout: ST[j, i] = sum_c k[c,j] q[c,i], computed as
    PE matmuls with k-tiles stationary and q moving.  Matmul operands are
    fp16 (fp32 matmul costs 4x on the PE; fp16 runs at bf16 speed with 8x
    finer rounding, and all operands are O(1) after GroupNorm so fp16 range
    is safe; accumulation stays fp32 in PSUM).
    Softmax runs WITHOUT max subtraction (scores are ~N(0,1) after GroupNorm,
    so fp32 exp cannot overflow); exp runs on ACT fused with the 1/sqrt(c)
    scale, PSUM -> SBUF.
  - The value/projection matmuls are fused: w = (Wp @ Wv) @ xn is precomputed
    in [n, c] layout with an extra all-ones column, so the PV matmul
    accumulates both the projected attention output AND the softmax
    denominators (row sums) in one PSUM tile.  Biases collapse to a single
    per-channel epilogue bias (Wp@bv + bp) because softmax rows sum to 1.
  - k/q live duplicated on SBUF partitions 0-63 / 64-127 so ST matmul pairs
    run concurrently in distinct PE row groups (K=64 row packing).
  - Chunk emission is software-pipelined (ST(c+1) is emitted before PV(c)) so
    the PE works on the next chunk's scores while ACT exponentiates.
"""

import numpy as np
import ml_dtypes

import concourse.bacc as bacc
import concourse.bass as bass
import concourse.tile as tile
from concourse import mybir
from concourse.bass_utils import run_bass_kernel_spmd

F32 = mybir.dt.float32
MMDT = mybir.dt.float16  # matmul operand dtype: fp16 = bf16 speed, 8x less rounding
AF = mybir.ActivationFunctionType
ALU = mybir.AluOpType

C = 64          # channels
N = 4096        # h*w
NQ = 2048       # query columns per core
NB = 4          # query blocks of 512
NJ = 32         # j tiles of 128
QB = 512        # query block width
JT = 128        # j tile width
NGROUPS = 8
EPS = 1e-5
GSIZE = C // NGROUPS * N  # elements per (batch, group) = 32768

# packed constants layout (columns in the "cpack" [64, 142] fp32 input)
CP_WV = 0        # Wv (for W2T = Wv^T @ Wp^T on PE)
CP_WPT = 64      # Wp^T
CP_G = 128       # group indicator G [64, 8]
CP_BQ = 136
CP_BK = 137
CP_BV = 138
CP_BP = 139
CP_GAMMA = 140
CP_BETA = 141
CP_GT = 142      # G^T [8, 64] on partitions 0..7 (PE group->channel broadcast)
CP_WQT = 206     # Wq^T fp32 (for exact folded biases)
CP_WKT = 270     # Wk^T fp32
CP_COLS = 334

# j-chunk schedule: alternating 4-tile / 3-tile chunks (4+3 PSUM banks for the
# two ST ping-pong buffers + 1 bank for the PV accumulator = 8 banks total).
CHUNKS = [(0, 4), (4, 3), (7, 4), (11, 3), (14, 4), (18, 3), (21, 4), (25, 3),
          (28, 2), (30, 2)]


def build_bass(stage=5, reps=1):
    """stage: 1=groupnorm only, 2=+projections, 3=+ST/exp, 4=+PV, 5=full.

    reps > 1 repeats the whole computation back-to-back in one NEFF (for
    wall-clock-difference benchmarking)."""
    nc = bacc.Bacc("TRN2", target_bir_lowering=False, debug=False, num_devices=8)
    _emit(nc, stage, reps)
    nc.compile()
    return nc


def _emit_pv(nc, pv_ps, wg, jt0, cn, e_s):
    for t in range(cn):
        jt = jt0 + t
        nc.tensor.matmul(
            out=pv_ps,
            lhsT=wg[jt // 8][:, jt % 8, :],
            rhs=e_s[:, t * QB:(t + 1) * QB],
            start=(jt == 0), stop=(jt == NJ - 1),
            skip_group_check=True,
        )


def _emit(nc, stage, reps=1):
    xb = nc.dram_tensor("xb", [C, N], F32, kind="ExternalInput")
    xq = nc.dram_tensor("xq", [C, NQ], F32, kind="ExternalInput")
    cpack = nc.dram_tensor("cpack", [C, CP_COLS], F32, kind="ExternalInput")
    wqtb_d = nc.dram_tensor("wqtb", [C, C], MMDT, kind="ExternalInput")
    wktb_d = nc.dram_tensor("wktb", [C, C], MMDT, kind="ExternalInput")
    out_d = nc.dram_tensor("out", [C, NQ], F32, kind="ExternalOutput")

    # internal DRAM scratch for the row-sum partition broadcast
    rsc = [nc.dram_tensor(f"rsc{b}", [1, QB], F32) for b in range(NB)]

    with tile.TileContext(nc) as tc:
        with (
            tc.tile_pool(name="consts", bufs=1) as consts,
            tc.tile_pool(name="big", bufs=1) as big,
            tc.tile_pool(name="epool", bufs=3) as epool,
            tc.tile_pool(name="small", bufs=2) as small,
            tc.tile_pool(name="ps_a", bufs=1, space="PSUM") as ps_a,
            tc.tile_pool(name="ps_b", bufs=1, space="PSUM") as ps_b,
            tc.tile_pool(name="ps_pv", bufs=1, space="PSUM") as ps_pv,
        ):
          for _rep in range(reps):
            # dummy Ln: triggers the one ACT table load (natural_log_exp set
            # covers Ln+Exp) while the input DMAs are still in flight
            warm = consts.tile([1, 1], F32, tag="warm")
            nc.vector.memset(warm, 1.0)
            ones_r = consts.tile([C + 1, C], F32, tag="onesr")
            nc.vector.memset(ones_r[C:C + 1, :], 1.0)
            nc.scalar.activation(out=warm, in_=warm, func=AF.Exp, bias=0.0, scale=1.0)

            # ---- inputs (x first: it gates the stats critical path; two
            # tiles so stats/casts start when the first half lands) ----
            x_a = big.tile([C, N // 2], F32, tag="xa")
            x_b = big.tile([C, N // 2], F32, tag="xb")
            xq_s = big.tile([C, NQ], F32, tag="xq")
            nc.sync.dma_start(out=x_a, in_=xb[:, 0:N // 2])
            nc.gpsimd.dma_start(out=x_b, in_=xb[:, N // 2:])
            nc.sync.dma_start(out=xq_s, in_=xq[:, :])
            cp = consts.tile([C, CP_COLS], F32, tag="cp")
            wqtb = consts.tile([C, C], MMDT, tag="wqtb")
            wktb = consts.tile([C, C], MMDT, tag="wktb")
            nc.sync.dma_start(out=cp, in_=cpack[:, :])
            nc.sync.dma_start(out=wqtb, in_=wqtb_d[:, :])
            nc.sync.dma_start(out=wktb, in_=wktb_d[:, :])

            # W2T / fused bias only depend on cp: compute while stats run
            w2t_ps = ps_pv.tile([C, C], F32, tag="pv")
            nc.tensor.matmul(out=w2t_ps, lhsT=cp[:, CP_WV:CP_WV + C],
                             rhs=cp[:, CP_WPT:CP_WPT + C], start=True, stop=True)
            w2t_s = big.tile([C, C], MMDT, tag="w2t")
            nc.vector.tensor_copy(out=w2t_s, in_=w2t_ps)
            w2t_f = big.tile([C, C], F32, tag="w2tf")
            nc.vector.tensor_copy(out=w2t_f, in_=w2t_ps)
            b2_ps = ps_pv.tile([C, 1], F32, tag="pv")
            nc.tensor.matmul(out=b2_ps, lhsT=cp[:, CP_WPT:CP_WPT + C],
                             rhs=cp[:, CP_BV:CP_BV + 1], start=True, stop=True)
            btot = big.tile([C, 1], F32, tag="btot")
            nc.vector.tensor_add(out=btot, in0=b2_ps, in1=cp[:, CP_BP:CP_BP + 1])

            # ---- GroupNorm stats: per-channel sum / sumsq, split into x
            # halves so work starts as soon as the first half-DMA lands ----
            scr_a = big.tile([C, N // 2], F32, tag="scra")
            scr_b = big.tile([C, N // 2], F32, tag="scrb")
            s12 = big.tile([C, 2], F32, tag="s12")
            s12h = big.tile([C, 4], F32, tag="s12h")
            nc.scalar.activation(out=scr_a, in_=x_a,
                                 func=AF.Square, accum_out=s12h[:, 2:3])
            nc.vector.reduce_sum(out=s12h[:, 0:1], in_=x_a,
                                 axis=mybir.AxisListType.X)
            nc.scalar.activation(out=scr_b, in_=x_b,
                                 func=AF.Square, accum_out=s12h[:, 3:4])
            nc.vector.reduce_sum(out=s12h[:, 1:2], in_=x_b,
                                 axis=mybir.AxisListType.X)
            nc.vector.tensor_add(out=s12[:, 0:1], in0=s12h[:, 0:1],
                                 in1=s12h[:, 1:2])
            nc.vector.tensor_add(out=s12[:, 1:2], in0=s12h[:, 2:3],
                                 in1=s12h[:, 3:4])
            # fp16 casts of RAW x/xq: GroupNorm is folded into the projection
            # weights, so these do not wait for the stats chain; xb16 is two
            # tiles so each projection waits only on its own half
            xb16a = big.tile([C, N // 2], MMDT, tag="xb16a")
            xb16b = big.tile([C, N // 2], MMDT, tag="xb16b")
            xq16 = big.tile([C, NQ], MMDT, tag="xq16")
            nc.scalar.activation(out=xb16a, in_=x_a, func=AF.Copy)
            nc.scalar.activation(out=xb16b, in_=x_b, func=AF.Copy)
            nc.vector.tensor_copy(out=xq16, in_=xq_s)
            gstat = ps_pv.tile([NGROUPS, 2], F32, tag="pv")
            nc.tensor.matmul(out=gstat, lhsT=cp[:, CP_G:CP_G + NGROUPS], rhs=s12,
                             start=True, stop=True)

            # per-group mean / E[x^2] -> var -> rstd = exp(-0.5*ln(var+eps))
            tmv = big.tile([NGROUPS, 2], F32, tag="tmv")
            nc.vector.tensor_scalar_mul(out=tmv, in0=gstat, scalar1=1.0 / GSIZE)
            var = big.tile([NGROUPS, 1], F32, tag="var")
            nc.vector.tensor_mul(out=var, in0=tmv[:, 0:1], in1=tmv[:, 0:1])
            nc.vector.tensor_sub(out=var, in0=tmv[:, 1:2], in1=var)
            # rstd = rsqrt(var+eps) via bit-trick seed + 3 Newton iterations,
            # entirely on the DVE (no ACT table set switches)
            tgrp = big.tile([NGROUPS, 2], F32, tag="tgrp")
            veps = big.tile([NGROUPS, 1], F32, tag="veps")
            vh = big.tile([NGROUPS, 1], F32, tag="vh")
            nc.vector.tensor_scalar_add(out=veps, in0=var, scalar1=EPS)
            nc.vector.tensor_scalar_mul(out=vh, in0=veps, scalar1=0.5)
            magic = consts.tile([NGROUPS, 1], mybir.dt.int32, tag="magic")
            nc.vector.memset(magic, 0x5F3759DF)
            c15 = consts.tile([NGROUPS, 1], F32, tag="c15")
            nc.vector.memset(c15, 1.5)
            y_i = big.tile([NGROUPS, 1], mybir.dt.int32, tag="yi")
            nc.vector.tensor_scalar(
                out=y_i, in0=veps.bitcast(mybir.dt.int32), scalar1=1, scalar2=None,
                op0=ALU.arith_shift_right,
            )
            nc.vector.tensor_sub(out=y_i, in0=magic, in1=y_i)
            y_f = y_i.bitcast(F32)
            t_n = big.tile([NGROUPS, 1], F32, tag="tn")
            for _it in range(3):
                nc.vector.tensor_mul(out=t_n, in0=y_f, in1=y_f)
                nc.vector.tensor_mul(out=t_n, in0=t_n, in1=vh)
                nc.vector.scalar_tensor_tensor(
                    out=t_n, in0=t_n, scalar=-1.0, in1=c15,
                    op0=ALU.mult, op1=ALU.add,
                )
                nc.vector.tensor_mul(out=y_f, in0=y_f, in1=t_n)
            nc.vector.tensor_copy(out=tgrp[:, 0:1], in_=y_f)
            nc.vector.tensor_copy(out=tgrp[:, 1:2], in_=tmv[:, 0:1])

            # expand [8,2] group stats -> [64,2] per-channel on the PE
            # (gexp[c] = tgrp[c//8] via the G^T indicator as stationary)
            gexp_ps = ps_pv.tile([C, 2], F32, tag="pv")
            nc.tensor.matmul(out=gexp_ps, lhsT=cp[0:NGROUPS, CP_GT:CP_GT + C],
                             rhs=tgrp, start=True, stop=True)

            a_s = big.tile([C, 1], F32, tag="a")
            b_s = big.tile([C, 1], F32, tag="b")
            nc.vector.tensor_mul(out=a_s, in0=gexp_ps[:, 0:1],
                                 in1=cp[:, CP_GAMMA:CP_GAMMA + 1])
            nc.vector.tensor_mul(out=b_s, in0=gexp_ps[:, 1:2], in1=a_s)
            nc.vector.tensor_sub(out=b_s, in0=cp[:, CP_BETA:CP_BETA + 1], in1=b_s)

            # fold GroupNorm into the projections: W' = W*diag(A) (scale the
            # c_in partition of the stored W^T by A), bias' = W@B + bias
            wq2 = big.tile([C, C], MMDT, tag="wq2")
            wk2 = big.tile([C, C], MMDT, tag="wk2")
            w2t2 = big.tile([C, C], MMDT, tag="w2t2")
            nc.vector.tensor_scalar_mul(out=wq2, in0=wqtb, scalar1=a_s)
            nc.vector.tensor_scalar_mul(out=wk2, in0=wktb, scalar1=a_s)
            nc.vector.tensor_scalar_mul(out=w2t2, in0=w2t_s, scalar1=a_s)
            bqp_ps = ps_pv.tile([C, 1], F32, tag="pv")
            nc.tensor.matmul(out=bqp_ps, lhsT=cp[:, CP_WQT:CP_WQT + C], rhs=b_s,
                             start=True, stop=True)
            bqp = big.tile([C, 1], F32, tag="bqp")
            nc.vector.tensor_add(out=bqp, in0=bqp_ps, in1=cp[:, CP_BQ:CP_BQ + 1])
            bkp_ps = ps_pv.tile([C, 1], F32, tag="pv")
            nc.tensor.matmul(out=bkp_ps, lhsT=cp[:, CP_WKT:CP_WKT + C], rhs=b_s,
                             start=True, stop=True)
            bkp = big.tile([C, 1], F32, tag="bkp")
            nc.vector.tensor_add(out=bkp, in0=bkp_ps, in1=cp[:, CP_BK:CP_BK + 1])
            # w loses its W2@B term (constant over j); softmax rows sum to 1,
            # so it lands in the epilogue bias: btot2 = btot + W2@B
            b2b_ps = ps_pv.tile([C, 1], F32, tag="pv")
            nc.tensor.matmul(out=b2b_ps, lhsT=w2t_f, rhs=b_s,
                             start=True, stop=True)
            btot2 = big.tile([C, 1], F32, tag="btot2")
            nc.vector.tensor_add(out=btot2, in0=b2b_ps, in1=btot)
            if stage <= 1:
                o1 = big.tile([C, NQ], F32, tag="dbg1")
                nc.vector.tensor_scalar(
                    out=o1, in0=xq_s, scalar1=a_s, scalar2=b_s,
                    op0=ALU.mult, op1=ALU.add,
                )
                nc.sync.dma_start(out=out_d[:, :], in_=o1)
                return

            # ---- projections: q2 [128, NQ], k2 split into two [128, N/2]
            #      tiles, w_aug split into 4 group tiles (finer tile deps let
            #      the first attention chunks start before setup finishes) ----
            q2t = []
            for b in range(NB):
                q2_b = big.tile([2 * C, QB], MMDT, tag=f"q2{b}")
                q2t.append(q2_b)
            k2aa = big.tile([2 * C, 3 * QB], MMDT, tag="k2aa")
            k2ab = big.tile([2 * C, QB], MMDT, tag="k2ab")
            k2b = big.tile([2 * C, N // 2], MMDT, tag="k2b")
            wg = []
            for g in range(4):
                wg_t = big.tile([2 * C, 8, C + 1], MMDT, tag=f"waug{g}")
                wg.append(wg_t)
            for g in range(4):
                nc.vector.memset(wg[g][:, :, C:C + 1], 1.0)

            qp = ps_a.tile([C, 2 * QB], F32, tag="a1")
            qp2 = ps_a.tile([C, 2 * QB], F32, tag="a2")
            for s in range(4):
                dst = qp if s < 2 else qp2
                nc.tensor.matmul(
                    out=dst[:, (s % 2) * QB:(s % 2 + 1) * QB], lhsT=wq2,
                    rhs=xq16[:, s * QB:(s + 1) * QB],
                    start=True, stop=True,
                )
            for b in range(NB):
                src_ps = qp if b < 2 else qp2
                nc.vector.tensor_scalar_add(
                    out=q2t[b][0:C, :], in0=src_ps[:, (b % 2) * QB:(b % 2 + 1) * QB],
                    scalar1=bqp,
                )
                nc.sync.dma_start(out=q2t[b][C:2 * C, :], in_=q2t[b][0:C, :])

            def k_group(dst, pool, tg, src0, gn, doff):
                kp = pool.tile([C, gn * QB], F32, tag=tg)
                for s in range(gn):
                    col = (src0 + s) * QB
                    xsrc = xb16a if col < N // 2 else xb16b
                    coff = col if col < N // 2 else col - N // 2
                    nc.tensor.matmul(
                        out=kp[:, s * QB:(s + 1) * QB], lhsT=wk2,
                        rhs=xsrc[:, coff:coff + QB],
                        start=True, stop=True,
                    )
                nc.vector.tensor_scalar_add(
                    out=dst[0:C, doff * QB:(doff + gn) * QB], in0=kp,
                    scalar1=bkp,
                )

            def w_group(g, pool, tg):
                wp_ps = pool.tile([JT, 8 * C], F32, tag=tg)
                for t in range(8):
                    jt = 8 * g + t
                    col = jt * JT
                    xsrc = xb16a if col < N // 2 else xb16b
                    coff = col if col < N // 2 else col - N // 2
                    nc.tensor.matmul(
                        out=wp_ps[:, t * C:(t + 1) * C],
                        lhsT=xsrc[:, coff:coff + JT], rhs=w2t2,
                        start=True, stop=True,
                    )
                nc.vector.tensor_copy(out=wg[g][:, :, 0:C], in_=wp_ps)

            k_group(k2aa, ps_b, "b1", 0, 3, 0)
            nc.sync.dma_start(out=k2aa[C:2 * C, :], in_=k2aa[0:C, :])
            k_group(k2ab, ps_a, "a1", 3, 1, 0)
            nc.sync.dma_start(out=k2ab[C:2 * C, :], in_=k2ab[0:C, :])
            w_group(0, ps_b, "b1")
            anchor = tc.tile_snap_priority()
            k_group(k2b, ps_a, "a2", 4, 2, 0)
            k_group(k2b, ps_a, "a2", 6, 2, 2)
            nc.sync.dma_start(out=k2b[C:2 * C, :], in_=k2b[0:C, :])
            w_group(1, ps_b, "b1")
            w_group(2, ps_a, "a1")
            w_group(3, ps_b, "b1")

            if stage == 2:
                o2 = big.tile([C, NQ], F32, tag="dbg1")
                nc.vector.tensor_copy(out=o2, in_=k2b[0:C, 0:NQ])
                nc.sync.dma_start(out=out_d[:, :], in_=o2)
                return

            # ---- attention ----
            # Software-pipelined emission: chunk c's ST matmuls and exp are
            # emitted before chunk c-1's PV matmuls, so the PE's static
            # instruction order lets ST(c+1) run while ACT computes exp(c).
            for b in range(NB if stage >= 5 else 1):
                import contextlib
                prio = (tc.high_priority(offset=tc.cur_priority - anchor)
                        if b == 0 else contextlib.nullcontext())
                with prio:
                  pv_ps = ps_pv.tile([C + 1, QB], F32, tag="pv")
                  pending_pv = None  # (jt0, cn, e_s) of previous chunk
                  for ci, (jt0, cn) in enumerate(CHUNKS):
                    pool, tg = (ps_a, "a") if ci % 2 == 0 else (ps_b, "b")
                    # 4-tile chunks use two 2-bank PSUM tiles + two exps so the
                    # first exp starts after only 2 score matmuls
                    if cn == 4:
                        st_h1 = pool.tile([2 * C, 2 * QB], F32, tag=tg + "1")
                        st_h2 = pool.tile([2 * C, 2 * QB], F32, tag=tg + "2")
                        st_parts = [(st_h1, 0), (st_h2, 2)]
                    else:
                        st_h1 = pool.tile([2 * C, cn * QB], F32, tag=tg + "1")
                        st_parts = [(st_h1, 0)]
                    e_s = epool.tile([2 * C, 4 * QB], MMDT, tag="e")
                    for st_ps, t0 in st_parts:
                        pn = 2 if cn == 4 else cn
                        for t in range(t0, t0 + pn):
                            jt = jt0 + t
                            ro = C if (t % 2 == 1) else 0
                            if jt < 12:
                                ksrc, kj = k2aa, jt
                            elif jt < 16:
                                ksrc, kj = k2ab, jt - 12
                            else:
                                ksrc, kj = k2b, jt - 16
                            nc.tensor.matmul(
                                out=st_ps[:, (t - t0) * QB:(t - t0 + 1) * QB],
                                lhsT=ksrc[ro:ro + C, kj * JT:(kj + 1) * JT],
                                rhs=q2t[b][ro:ro + C, :],
                                start=True, stop=True,
                            )
                        nc.scalar.activation(
                            out=e_s[:, t0 * QB:(t0 + pn) * QB], in_=st_ps,
                            func=AF.Exp, scale=0.125,
                        )
                    if stage >= 4:
                        if pending_pv is not None:
                            _emit_pv(nc, pv_ps, wg, *pending_pv)
                        pending_pv = (jt0, cn, e_s)
                  if stage >= 4 and pending_pv is not None:
                    _emit_pv(nc, pv_ps, wg, *pending_pv)

                if stage == 3:
                    dbg = small.tile([C, QB], F32, tag="dbg")
                    nc.vector.tensor_copy(out=dbg, in_=e_s[0:C, 0:QB])
                    nc.sync.dma_start(out=out_d[:, 0:QB], in_=dbg)
                    continue
                # epilogue: divide by row sums, add bias, residual, store.
                # The [1,512] sums row (partition 64) is broadcast to all 64
                # channel partitions with a K=1 PE matmul against a ones row.
                pv_sb = small.tile([C + 1, QB], F32, tag="pvsb")
                nc.vector.tensor_copy(out=pv_sb, in_=pv_ps)
                sb_ps = ps_pv.tile([C, QB], F32, tag="pv")
                nc.tensor.matmul(out=sb_ps, lhsT=ones_r[C:C + 1, 0:C],
                                 rhs=pv_sb[C:C + 1, :], start=True, stop=True)
                rb_s = small.tile([C, QB], F32, tag="rb")
                nc.vector.reciprocal(out=rb_s, in_=sb_ps)
                o_s = small.tile([C, QB], F32, tag="o")
                nc.vector.tensor_mul(out=o_s, in0=pv_sb[0:C, :], in1=rb_s)
                nc.vector.scalar_tensor_tensor(
                    out=o_s, in0=o_s, scalar=btot2,
                    in1=xq_s[:, b * QB:(b + 1) * QB],
                    op0=ALU.add, op1=ALU.add,
                )
                nc.sync.dma_start(out=out_d[:, b * QB:(b + 1) * QB], in_=o_s)


_NC = None


def _get_nc():
    global _NC
    if _NC is None:
        _NC = build_bass()
    return _NC


def make_in_maps(x, gamma, beta, Wq, bq, Wk, bk, Wv, bv, Wp, bp):
    x = np.asarray(x, np.float32)
    b, c, h, w = x.shape
    n = h * w
    xf = np.ascontiguousarray(x.reshape(b, c, n))
    cpk = np.zeros((C, CP_COLS), np.float32)
    cpk[:, CP_WV:CP_WV + C] = np.asarray(Wv, np.float32)
    cpk[:, CP_WPT:CP_WPT + C] = np.asarray(Wp, np.float32).T
    cpk[np.arange(C), CP_G + np.arange(C) // (C // NGROUPS)] = 1.0
    for col, v in [(CP_BQ, bq), (CP_BK, bk), (CP_BV, bv), (CP_BP, bp),
                   (CP_GAMMA, gamma), (CP_BETA, beta)]:
        cpk[:, col] = np.asarray(v, np.float32)
    cpk[0:NGROUPS, CP_GT:CP_GT + C] = cpk[:, CP_G:CP_G + NGROUPS].T
    cpk[:, CP_WQT:CP_WQT + C] = np.asarray(Wq, np.float32).T
    cpk[:, CP_WKT:CP_WKT + C] = np.asarray(Wk, np.float32).T
    common = {
        "cpack": cpk,
        "wqtb": np.ascontiguousarray(
            np.asarray(Wq, np.float32).T.astype(np.float16)),
        "wktb": np.ascontiguousarray(
            np.asarray(Wk, np.float32).T.astype(np.float16)),
    }
    in_maps = []
    for core in range(8):
        bi, hi = divmod(core, 2)
        m = dict(common)
        m["xb"] = xf[bi]
        m["xq"] = np.ascontiguousarray(xf[bi][:, hi * NQ:(hi + 1) * NQ])
        in_maps.append(m)
    return in_maps


def assemble_out(results, b=4, c=64, h=64, w=64):
    n = h * w
    out = np.empty((b, c, n), np.float32)
    for core in range(8):
        bi, hi = divmod(core, 2)
        out[bi][:, hi * NQ:(hi + 1) * NQ] = results[core]["out"]
    return out.reshape(b, c, h, w)


def kernel(x, gamma, beta, Wq, bq, Wk, bk, Wv, bv, Wp, bp):
    nc = _get_nc()
    in_maps = make_in_maps(x, gamma, beta, Wq, bq, Wk, bk, Wv, bv, Wp, bp)
    res = run_bass_kernel_spmd(nc, in_maps, core_ids=list(range(8)))
    return assemble_out(res.results)

